# revision 2
# baseline (speedup 1.0000x reference)
"""Trainium2 Bass kernel for SAGAN-style attention (softmax over query axis).

Reference computation (per batch b):
    q = Wq @ xf + bq      [A, N]   A=32, N=4096
    k = Wk @ xf + bk      [A, N]
    v = Wv @ xf + bv      [C, N]   C=256
    e[i,j]    = sum_a q[a,i] k[a,j]
    attn      = softmax(e, axis=i)          (column-normalized over i)
    out[c,i]  = sum_j v[c,j] attn[i,j]
    y         = gamma * out + x

Sharding: 8 cores = 4 batches x 2 j-halves.  Each core computes, for its
batch and its half of the j (key) axis, the partial
    out_h[c,i] = sum_{j in half} (v[c,j]/Z[j]) * exp(e[i,j])
with Z[j] = sum_i exp(e[i,j]) (full i range lives on-core, so Z is local).
Host sums the two halves, multiplies gamma, adds x.

Device layout notes:
  - eT is computed transposed ([j,i] on [partition, free]) so the softmax
    normalization axis (i) is the free axis: ScalarE Exp + accum_out gives
    unnormalized P^T and the row-sum Z in one pass.  No max subtraction:
    |e| < ~40 so exp stays comfortably inside fp32/bf16 range.
  - The j-half is permuted to the front of the i axis on host (np.roll) so
    the SPMD program can use fixed column offsets; host un-rolls the output.
"""

import numpy as np
import ml_dtypes

B = 4
C = 256
A = 32
N = 4096          # i (query) axis, full
NJ = 2048         # local j (key) axis per core
P = 128
NIT = 8           # i tiles of 512
NJT = 16          # local j tiles of 128
NCORES = 8

_CACHE = {}


def _build_nc():
    from contextlib import ExitStack
    import concourse.tile as tile
    from concourse import bacc, mybir

    dt = mybir.dt
    F32, BF16 = dt.float32, dt.bfloat16
    AF = mybir.ActivationFunctionType

    nc = bacc.Bacc("TRN2", target_bir_lowering=False, debug=False)

    xf_d = nc.dram_tensor("xf", [C, N], BF16, kind="ExternalInput").ap()
    wq_d = nc.dram_tensor("wqT", [C, A], BF16, kind="ExternalInput").ap()
    wk_d = nc.dram_tensor("wkT", [C, A], BF16, kind="ExternalInput").ap()
    wv_d = nc.dram_tensor("wvT", [C, C], BF16, kind="ExternalInput").ap()
    bq_d = nc.dram_tensor("bq", [A, 1], F32, kind="ExternalInput").ap()
    bk_d = nc.dram_tensor("bk", [A, 1], F32, kind="ExternalInput").ap()
    bv_d = nc.dram_tensor("bvrep", [P, C], F32, kind="ExternalInput").ap()
    out_d = nc.dram_tensor("out", [C, N], F32, kind="ExternalOutput").ap()

    with tile.TileContext(nc) as tc, ExitStack() as ctx:
        persist = ctx.enter_context(tc.tile_pool(name="persist", bufs=1))
        ostage = ctx.enter_context(tc.tile_pool(name="ostage", bufs=4))
        psum = ctx.enter_context(tc.tile_pool(name="psum", bufs=2, space="PSUM"))

        # ---- persistent SBUF tensors
        xf_s = [persist.tile([P, N], BF16, name=f"xf{cc}", tag=f"xf{cc}") for cc in range(2)]
        wq_s = [persist.tile([P, A], BF16, name=f"wq{cc}", tag=f"wq{cc}") for cc in range(2)]
        wk_s = [persist.tile([P, A], BF16, name=f"wk{cc}", tag=f"wk{cc}") for cc in range(2)]
        wv_s = [persist.tile([P, C], BF16, name=f"wv{cc}", tag=f"wv{cc}") for cc in range(2)]
        bq_s = persist.tile([A, 1], F32, name="bq", tag="bq")
        bk_s = persist.tile([A, 1], F32, name="bk", tag="bk")
        bv_s = persist.tile([P, C], F32, name="bv", tag="bv")
        q_s = persist.tile([A, N], BF16, name="q", tag="q")
        k_s = persist.tile([A, NJ], BF16, name="k", tag="k")
        vt_s = persist.tile([P, NJT * C], F32, name="vt", tag="vt")
        pt_s = persist.tile([P, NJT * N], BF16, name="pt", tag="pt")
        zp_s = persist.tile([P, 2 * NJT], F32, name="zp", tag="zp")
        zs_s = persist.tile([P, NJT], F32, name="zs", tag="zs")
        rz_s = persist.tile([P, NJT], F32, name="rz", tag="rz")
        vh_s = persist.tile([P, NJT * C], BF16, name="vh", tag="vh")

        # ---- input DMA
        for cc in range(2):
            nc.sync.dma_start(out=xf_s[cc][:], in_=xf_d[cc * P:(cc + 1) * P, :])
            nc.sync.dma_start(out=wq_s[cc][:], in_=wq_d[cc * P:(cc + 1) * P, :])
            nc.sync.dma_start(out=wk_s[cc][:], in_=wk_d[cc * P:(cc + 1) * P, :])
            nc.sync.dma_start(out=wv_s[cc][:], in_=wv_d[cc * P:(cc + 1) * P, :])
        nc.sync.dma_start(out=bq_s[:], in_=bq_d[:])
        nc.sync.dma_start(out=bk_s[:], in_=bk_d[:])
        nc.sync.dma_start(out=bv_s[:], in_=bv_d[:])

        # ---- projections
        # q[a,i]: lhsT=WqT chunk [128c, 32a], rhs=xf chunk [128c, 512i]
        for it in range(NIT):
            pq = psum.tile([A, 512], F32, name="ps", tag="ps")
            for cc in range(2):
                nc.tensor.matmul(pq[:], lhsT=wq_s[cc][:],
                                 rhs=xf_s[cc][:, it * 512:(it + 1) * 512],
                                 start=(cc == 0), stop=(cc == 1))
            nc.vector.tensor_scalar_add(q_s[:, it * 512:(it + 1) * 512],
                                        pq[:], bq_s[:, 0:1])
        # k[a,j]: same, over local j (= first NJ columns of permuted xf)
        for jc in range(NJ // 512):
            pk = psum.tile([A, 512], F32, name="ps", tag="ps")
            for cc in range(2):
                nc.tensor.matmul(pk[:], lhsT=wk_s[cc][:],
                                 rhs=xf_s[cc][:, jc * 512:(jc + 1) * 512],
                                 start=(cc == 0), stop=(cc == 1))
            nc.vector.tensor_scalar_add(k_s[:, jc * 512:(jc + 1) * 512],
                                        pk[:], bk_s[:, 0:1])
        # vT[j,c]: lhsT=xf chunk [128c, 128j], rhs=WvT chunk [128c, 256c_out]
        for jt in range(NJT):
            pv = psum.tile([P, C], F32, name="ps", tag="ps")
            for cc in range(2):
                nc.tensor.matmul(pv[:], lhsT=xf_s[cc][:, jt * P:(jt + 1) * P],
                                 rhs=wv_s[cc][:],
                                 start=(cc == 0), stop=(cc == 1))
            nc.vector.tensor_add(vt_s[:, jt * C:(jt + 1) * C], pv[:], bv_s[:])

        # ---- energy + exp (phase A)
        # eT[j,i] = sum_a k[a,j] q[a,i]: lhsT=k tile [32a, 128j], rhs=q [32a, 512i]
        for jt in range(NJT):
            for r in range(2):
                pe = psum.tile([P, 2048], F32, name="ps", tag="ps")
                for s in range(4):
                    i0 = r * 2048 + s * 512
                    nc.tensor.matmul(pe[:, s * 512:(s + 1) * 512],
                                     lhsT=k_s[:, jt * P:(jt + 1) * P],
                                     rhs=q_s[:, i0:i0 + 512],
                                     start=True, stop=True)
                o0 = jt * N + r * 2048
                nc.scalar.activation(pt_s[:, o0:o0 + 2048], pe[:], AF.Exp,
                                     accum_out=zp_s[:, r * NJT + jt:r * NJT + jt + 1])

        # ---- Z, 1/Z, vhat = vT/Z
        nc.vector.tensor_add(zs_s[:], zp_s[:, 0:NJT], zp_s[:, NJT:2 * NJT])
        nc.vector.reciprocal(rz_s[:], zs_s[:])
        for jt in range(NJT):
            nc.vector.tensor_scalar_mul(vh_s[:, jt * C:(jt + 1) * C],
                                        vt_s[:, jt * C:(jt + 1) * C],
                                        rz_s[:, jt:jt + 1])

        # ---- out partial (phase B)
        # out[c,i] = sum_j vhat[c,j] P^T[j,i]: lhsT=vhat tile [128j, 128c],
        # rhs=P^T tile [128j, 512i], accumulate over the 16 j tiles.
        for it in range(NIT):
            for cc in range(2):
                po = psum.tile([P, 512], F32, name="ps", tag="ps")
                for jt in range(NJT):
                    nc.tensor.matmul(
                        po[:],
                        lhsT=vh_s[:, jt * C + cc * P: jt * C + cc * P + P],
                        rhs=pt_s[:, jt * N + it * 512: jt * N + (it + 1) * 512],
                        start=(jt == 0), stop=(jt == NJT - 1))
                ot = ostage.tile([P, 512], F32, name="ot", tag="ot")
                if (it + cc) % 2 == 0:
                    nc.scalar.copy(ot[:], po[:])
                else:
                    nc.vector.tensor_copy(ot[:], po[:])
                nc.sync.dma_start(
                    out=out_d[cc * P:(cc + 1) * P, it * 512:(it + 1) * 512],
                    in_=ot[:])

    nc.compile()
    return nc


def _get_nc():
    if "nc" not in _CACHE:
        _CACHE["nc"] = _build_nc()
    return _CACHE["nc"]


def make_in_maps(x, Wq, bq, Wk, bk, Wv, bv):
    bf = ml_dtypes.bfloat16
    xf = np.asarray(x, np.float32).reshape(B, C, N)
    wq_t = np.ascontiguousarray(np.asarray(Wq, np.float32).T).astype(bf)
    wk_t = np.ascontiguousarray(np.asarray(Wk, np.float32).T).astype(bf)
    wv_t = np.ascontiguousarray(np.asarray(Wv, np.float32).T).astype(bf)
    bq_c = np.asarray(bq, np.float32).reshape(A, 1).copy()
    bk_c = np.asarray(bk, np.float32).reshape(A, 1).copy()
    bv_rep = np.ascontiguousarray(
        np.broadcast_to(np.asarray(bv, np.float32).reshape(1, C), (P, C)))
    in_maps = []
    for core in range(NCORES):
        b, jh = divmod(core, 2)
        xp = xf[b] if jh == 0 else np.roll(xf[b], -NJ, axis=1)
        in_maps.append({
            "xf": np.ascontiguousarray(xp).astype(bf),
            "wqT": wq_t, "wkT": wk_t, "wvT": wv_t,
            "bq": bq_c, "bk": bk_c, "bvrep": bv_rep,
        })
    return in_maps


def combine_outputs(outs, x, gamma):
    xf = np.asarray(x, np.float32).reshape(B, C, N)
    g = float(np.asarray(gamma).reshape(-1)[0])
    y = np.empty((B, C, N), np.float32)
    for b in range(B):
        o = outs[2 * b] + np.roll(outs[2 * b + 1], NJ, axis=1)
        y[b] = g * o + xf[b]
    return y.reshape(np.asarray(x).shape)


def run_on_hw(in_maps, trace=False):
    from concourse.bass_utils import run_bass_kernel_spmd
    nc = _get_nc()
    res = run_bass_kernel_spmd(nc, in_maps, core_ids=list(range(NCORES)),
                               trace=trace)
    return res


def kernel(x, Wq, bq, Wk, bk, Wv, bv, gamma):
    in_maps = make_in_maps(x, Wq, bq, Wk, bk, Wv, bv)
    res = run_on_hw(in_maps, trace=False)
    outs = [np.asarray(res.results[i]["out"], np.float32)
            for i in range(NCORES)]
    return combine_outputs(outs, x, gamma)
